# revision 8
# baseline (speedup 1.0000x reference)
"""GAT (3-layer, PyG-style) Trainium2 Bass kernel, sharded across 8 NeuronCores.

v3: dst-range graph-parallel sharding, fused per-tile pipeline.
- Layer-0's gather table h0 = X @ W0' is computed redundantly per core from
  the full (replicated) X, so layer 0 needs no collectives at all.
- Layers 1/2 exchange their 528-col h_ext tables via AllGathers chunked 5
  ways with strided output views, streamed during the previous layer's
  aggregation so only the last chunk's wire time is exposed.
- Per dst tile: dma_gather of in-edge source rows (lo/hi int16 split,
  per-tile static trimmed num_idxs), edge weights w = exp(lrelu(als+ald))
  with al_d per edge via host-precomputed S^T selection-matrix matmuls,
  numerator/denominator scatter-adds as S^T matmuls in PSUM, and the next
  layer's h_ext matmul fused in the same loop. ELU's "-1" is folded into
  the next layer's bias. Self-loops are analytic.

kernel(**inputs) takes FULL inputs, returns the FULL [N, 16] output.
"""

import sys

sys.path.insert(0, "/opt/trn_rl_repo")

import numpy as np

import concourse.bass as bass
import concourse.mybir as mybir
import concourse.tile as tile
from concourse import bacc
from concourse import bass_utils
from concourse.bass_interp import get_hw_module
from concourse.masks import make_identity
from concourse import library_config

F32 = mybir.dt.float32
BF = mybir.dt.bfloat16
I16 = mybir.dt.int16
import ml_dtypes
NPBF = ml_dtypes.bfloat16
P = 128


def real_cfg():
    R = 8
    N = 50000
    PER = N // R                      # 6250 nodes per core
    T = (PER + P - 1) // P            # 49 dst tiles per core
    return dict(
        R=R, N=N, PER=PER, T=T, NPAD=T * P,
        F_IN=128, HID=64, HEADS=8, N_CLASSES=16,
        NEG=0.2, SPLIT_T=31,
    )


ROWG = 640     # h_ext row width, layers 0/1: [512 feat | 8 al_s | 8 al_d | pad]
GROW = 640     # gather row width (256B multiple)
ROWG2 = 128    # layer-2 table: [16 out | 1 al_s | 1 al_d | pad]


# ---------------------------------------------------------------------------
# Host-side preprocessing
# ---------------------------------------------------------------------------

def _wrap16(flat):
    n = flat.shape[-1]
    w = flat.reshape(n // 16, 16).T
    return np.ascontiguousarray(np.tile(w, (8, 1)), np.int16)


def host_prepare(inputs, cfg):
    R, N, PER, T, NPAD = cfg["R"], cfg["N"], cfg["PER"], cfg["T"], cfg["NPAD"]
    F_IN, HID, HEADS, NCLS = cfg["F_IN"], cfg["HID"], cfg["HEADS"], cfg["N_CLASSES"]
    HC = HID * HEADS
    SPLIT_T = cfg["SPLIT_T"]
    LO = SPLIT_T * P
    HI = NPAD - LO

    x = np.asarray(inputs["x"], np.float32)
    ei = np.asarray(inputs["edge_index"])
    src = ei[0].astype(np.int64)
    dst = ei[1].astype(np.int64)

    core = dst // PER
    dloc = (dst - core * PER).astype(np.int64)
    sloc = (src % PER).astype(np.int64)
    srank = (src // PER).astype(np.int64)
    is_lo = sloc < LO
    tile_of = dloc // P

    cl = np.zeros((R, T), np.int64)
    ch = np.zeros((R, T), np.int64)
    np.add.at(cl, (core[is_lo], tile_of[is_lo]), 1)
    np.add.at(ch, (core[~is_lo], tile_of[~is_lo]), 1)
    n16_lo = ((cl.max(axis=0) + 15) // 16 * 16).astype(np.int64)
    n16_hi = ((ch.max(axis=0) + 15) // 16 * 16).astype(np.int64)
    blo = (n16_lo + P - 1) // P
    bhi = (n16_hi + P - 1) // P
    bt = blo + bhi
    boff = np.concatenate([[0], np.cumsum(bt)])
    TOTB = int(boff[-1])
    iolo = np.concatenate([[0], np.cumsum(n16_lo // 16)])
    iohi = np.concatenate([[0], np.cumsum(n16_hi // 16)])
    ILO_TOT = int(iolo[-1])
    IHI_TOT = int(iohi[-1])

    grow = np.where(is_lo, srank * LO + sloc, srank * HI + (sloc - LO))

    order = np.lexsort(((~is_lo).astype(np.int64), tile_of, core))
    g_s = grow[order]
    d_s = dloc[order]
    core_s = core[order]
    tile_s = tile_of[order]
    lo_s = is_lo[order]

    grp = core_s * (2 * T) + tile_s * 2 + (~lo_s).astype(np.int64)
    grp_start = np.searchsorted(grp, np.arange(R * T * 2), side="left")
    pos = np.arange(len(grp)) - grp_start[grp]

    ilo_all = np.zeros((R, ILO_TOT * 16), np.int16)
    ihi_all = np.zeros((R, IHI_TOT * 16), np.int16)
    lo_m = lo_s
    hi_m = ~lo_s
    ilo_all[core_s[lo_m], iolo[tile_s[lo_m]] * 16 + pos[lo_m]] = g_s[lo_m].astype(np.int16)
    ihi_all[core_s[hi_m], iohi[tile_s[hi_m]] * 16 + pos[hi_m]] = g_s[hi_m].astype(np.int16)

    S_all = np.zeros((R, P, TOTB * P), NPBF)
    St_all = np.zeros((R, P, TOTB * P), NPBF)
    slot = np.where(lo_m, pos, blo[tile_s] * P + pos)
    blk = boff[tile_s] + slot // P
    prt = slot % P
    dt_s = d_s - tile_s * P
    S_all[core_s, prt, blk * P + dt_s] = np.float32(1)
    St_all[core_s, dt_s, blk * P + prt] = np.float32(1)

    def wext(W, a_s, a_d, ncols):
        Fin = W.shape[0]
        H, C = a_s.shape
        Wr = W.reshape(Fin, H, C)
        We = np.zeros((Fin, ncols), np.float32)
        We[:, : H * C] = W
        We[:, H * C : H * C + H] = np.einsum("fhc,hc->fh", Wr, a_s)
        We[:, H * C + H : H * C + 2 * H] = np.einsum("fhc,hc->fh", Wr, a_d)
        return We

    W0e = wext(np.asarray(inputs["W0"], np.float32),
               np.asarray(inputs["a_s0"], np.float32),
               np.asarray(inputs["a_d0"], np.float32), ROWG)
    W1e = wext(np.asarray(inputs["W1"], np.float32),
               np.asarray(inputs["a_s1"], np.float32),
               np.asarray(inputs["a_d1"], np.float32), ROWG)
    W2e = wext(np.asarray(inputs["W2"], np.float32),
               np.asarray(inputs["a_s2"], np.float32),
               np.asarray(inputs["a_d2"], np.float32), ROWG2)

    def bext(b, ncols, Wfull):
        be = np.zeros((1, ncols), np.float32)
        be[0, : b.shape[0]] = b
        if Wfull is not None:
            be[0, :] -= Wfull.sum(axis=0)   # fold ELU's -1 into the bias
        return np.ascontiguousarray(np.broadcast_to(be, (P, ncols)))

    b0e = bext(np.asarray(inputs["b0"], np.float32), ROWG, None)
    b1e = bext(np.asarray(inputs["b1"], np.float32), ROWG, W1e)
    b2e = bext(np.asarray(inputs["b2"], np.float32), ROWG2, W2e)

    W0b = W0e.astype(NPBF)
    W1b = np.ascontiguousarray(
        W1e.reshape(4, P, ROWG).transpose(1, 0, 2)).astype(NPBF)
    W2b = np.ascontiguousarray(
        W2e.reshape(4, P, ROWG2).transpose(1, 0, 2)).astype(NPBF)

    # full node-feature matrix, feature-major, padded per-rank to NPAD
    xf = np.zeros((F_IN, R * NPAD), np.float32)
    for r in range(R):
        xf[:, r * NPAD : r * NPAD + PER] = x[r * PER : (r + 1) * PER].T
    xf = xf.astype(NPBF)

    in_maps = []
    for r in range(R):
        lo_w = np.zeros((P, ILO_TOT), np.int16)
        hi_w = np.zeros((P, IHI_TOT), np.int16)
        for t in range(T):
            lo_w[:, iolo[t] : iolo[t + 1]] = _wrap16(
                ilo_all[r, iolo[t] * 16 : iolo[t + 1] * 16])
            hi_w[:, iohi[t] : iohi[t + 1]] = _wrap16(
                ihi_all[r, iohi[t] * 16 : iohi[t + 1] * 16])
        xt = np.zeros((F_IN, NPAD), np.float32)
        xt[:, :PER] = x[r * PER : (r + 1) * PER].T
        in_maps.append({
            "xt0": xt.astype(NPBF),
            "xfull": xf,
            "w0e": W0b, "w1e": W1b, "w2e": W2b,
            "b0e": b0e, "b1e": b1e, "b2e": b2e,
            "ilo": lo_w, "ihi": hi_w,
            "smat": S_all[r], "stmat": St_all[r],
        })

    meta = dict(
        n16_lo=n16_lo.tolist(), n16_hi=n16_hi.tolist(),
        blo=blo.tolist(), bhi=bhi.tolist(), bt=bt.tolist(),
        boff=boff.tolist(), iolo=iolo.tolist(), iohi=iohi.tolist(),
        TOTB=TOTB, ILO_TOT=ILO_TOT, IHI_TOT=IHI_TOT,
        BMAX=int(bt.max()),
    )
    return in_maps, meta


# ---------------------------------------------------------------------------
# Device program
# ---------------------------------------------------------------------------

def build_gat_nc(cfg, meta):
    R, PER, T, NPAD = cfg["R"], cfg["PER"], cfg["T"], cfg["NPAD"]
    F_IN, HID, HEADS, NCLS = cfg["F_IN"], cfg["HID"], cfg["HEADS"], cfg["N_CLASSES"]
    NEG = cfg["NEG"]
    HC = HID * HEADS
    SPLIT_T = cfg["SPLIT_T"]
    LO = SPLIT_T * P
    HI = NPAD - LO
    n16_lo, n16_hi = meta["n16_lo"], meta["n16_hi"]
    blo_l, bhi_l, bt_l = meta["blo"], meta["bhi"], meta["bt"]
    boff, iolo, iohi = meta["boff"], meta["iolo"], meta["iohi"]
    TOTB, ILO_TOT, IHI_TOT = meta["TOTB"], meta["ILO_TOT"], meta["IHI_TOT"]
    BMAX = meta["BMAX"]
    ILOMAX = max(n16_lo) // 16
    IHIMAX = max(n16_hi) // 16

    nc = bacc.Bacc("TRN2", target_bir_lowering=False, debug=False,
                   num_devices=R)

    xfull_d = nc.dram_tensor("xfull", [F_IN, R * NPAD], BF, kind="ExternalInput")
    xt0_d = nc.dram_tensor("xt0", [F_IN, NPAD], BF, kind="ExternalInput")
    w0e_d = nc.dram_tensor("w0e", [P, ROWG], BF, kind="ExternalInput")
    w1e_d = nc.dram_tensor("w1e", [P, 4, ROWG], BF, kind="ExternalInput")
    w2e_d = nc.dram_tensor("w2e", [P, 4, ROWG2], BF, kind="ExternalInput")
    b0e_d = nc.dram_tensor("b0e", [P, ROWG], F32, kind="ExternalInput")
    b1e_d = nc.dram_tensor("b1e", [P, ROWG], F32, kind="ExternalInput")
    b2e_d = nc.dram_tensor("b2e", [P, ROWG2], F32, kind="ExternalInput")
    ilo_d = nc.dram_tensor("ilo", [P, ILO_TOT], I16, kind="ExternalInput")
    ihi_d = nc.dram_tensor("ihi", [P, IHI_TOT], I16, kind="ExternalInput")
    smat_d = nc.dram_tensor("smat", [P, TOTB * P], BF, kind="ExternalInput")
    stmat_d = nc.dram_tensor("stmat", [P, TOTB * P], BF, kind="ExternalInput")
    out_d = nc.dram_tensor("out", [PER, NCLS], F32, kind="ExternalOutput")

    rg = [list(range(R))]

    with tile.TileContext(nc) as tc:
        with (
            tc.tile_pool(name="pers", bufs=1) as pers,
            tc.tile_pool(name="sb", bufs=3) as sb,
            tc.tile_pool(name="sbg", bufs=4) as sbg,
            tc.tile_pool(name="psA", bufs=2, space="PSUM") as psA,
            tc.tile_pool(name="psB", bufs=2, space="PSUM") as psB,
            tc.tile_pool(name="psC", bufs=1, space="PSUM") as psC,
            tc.tile_pool(name="psD", bufs=2, space="PSUM") as psD,
            tc.tile_pool(name="psE", bufs=1, space="PSUM") as psE,
            tc.tile_pool(name="dram", bufs=1, space="DRAM") as dram,
        ):
            nc.gpsimd.load_library(library_config.mlp)

            ident = pers.tile([P, P], F32)
            make_identity(nc, ident[:])
            w0_sb = pers.tile([P, 1, ROWG], BF)
            w1_sb = pers.tile([P, 4, ROWG], BF)
            w2_sb = pers.tile([P, 4, ROWG2], BF)
            b0_sb = pers.tile([P, ROWG], F32)
            b1_sb = pers.tile([P, ROWG], F32)
            b2_sb = pers.tile([P, ROWG2], F32)
            nc.sync.dma_start(w0_sb[:, 0, :], w0e_d[:, :])
            nc.sync.dma_start(w1_sb[:], w1e_d[:, :, :])
            nc.sync.dma_start(w2_sb[:], w2e_d[:, :, :])
            nc.sync.dma_start(b0_sb[:], b0e_d[:, :])
            nc.sync.dma_start(b1_sb[:], b1e_d[:, :])
            nc.sync.dma_start(b2_sb[:], b2e_d[:, :])

            for _ in range(4):
                gg = sbg.tile([P, BMAX, GROW], BF, tag="g")
                nc.vector.memset(gg[:].rearrange("p a b -> p (a b)"), 0.0)
                gg2 = sbg.tile([P, BMAX, ROWG2], BF, tag="g2")
                nc.vector.memset(gg2[:].rearrange("p a b -> p (a b)"), 0.0)

            # gather tables (row width GROW; cols ROWG..GROW stay garbage)
            tlo = [dram.tile([R * LO, GROW], BF, name="tlo0"),
                   dram.tile([R * LO, GROW], BF, addr_space="Shared", name="tlo1"),
                   dram.tile([R * LO, ROWG2], BF, addr_space="Shared", name="tlo2")]
            thi = [dram.tile([R * HI, GROW], BF, name="thi0"),
                   dram.tile([R * HI, GROW], BF, addr_space="Shared", name="thi1"),
                   dram.tile([R * HI, ROWG2], BF, addr_space="Shared", name="thi2")]
            # local h_ext staging (input of the chunked AllGathers), L=1,2
            hlo = [None,
                   dram.tile([LO, ROWG], BF, name="hlo1"),
                   dram.tile([LO, ROWG2], BF, name="hlo2")]
            hhi = [None,
                   dram.tile([HI, ROWG], BF, name="hhi1"),
                   dram.tile([HI, ROWG2], BF, name="hhi2")]
            hown0 = dram.tile([NPAD, ROWG], BF, name="hown0")

            def ag_chunks(L, t, rowg_t):
                """Fire the layer-L table AllGathers (one per group)."""
                if t == SPLIT_T - 1:
                    nc.gpsimd.collective_compute(
                        "AllGather", mybir.AluOpType.bypass,
                        replica_groups=rg,
                        ins=[hlo[L][:, :]], outs=[tlo[L][:, :]])
                if t == T - 1:
                    nc.gpsimd.collective_compute(
                        "AllGather", mybir.AluOpType.bypass,
                        replica_groups=rg,
                        ins=[hhi[L][:, :]], outs=[thi[L][:, :]])

            def hext_store(L, t, hsb):
                """Store one tile's h_ext rows for layer L (L>=1) + chunks."""
                if t < SPLIT_T:
                    nc.sync.dma_start(hlo[L][t * P : (t + 1) * P, :], hsb[:])
                else:
                    r0 = t * P - LO
                    nc.sync.dma_start(hhi[L][r0 : r0 + P, :], hsb[:])
                ag_chunks(L, t, None)

            # ---- layer-0 own rows -> hown0 (rank-independent layout) ----
            for tl in range(T):
                ph = psD.tile([P, 512], F32, tag="ph")
                ph_hi = psE.tile([P, ROWG - 512], F32, tag="phh")
                xsl = sb.tile([P, P], BF, tag="xsl")
                nc.sync.dma_start(xsl[:], xt0_d[:, tl * P : (tl + 1) * P])
                nc.tensor.matmul(ph[:], lhsT=xsl[:], rhs=w0_sb[:, 0, 0:512],
                                 start=True, stop=True)
                nc.tensor.matmul(ph_hi[:], lhsT=xsl[:], rhs=w0_sb[:, 0, 512:ROWG],
                                 start=True, stop=True)
                hsb = sb.tile([P, ROWG], BF, tag="hsb640")
                nc.vector.tensor_tensor(hsb[:, 0:512], ph[:], b0_sb[:, 0:512],
                                        mybir.AluOpType.add)
                nc.vector.tensor_tensor(hsb[:, 512:ROWG], ph_hi[:],
                                        b0_sb[:, 512:ROWG], mybir.AluOpType.add)
                nc.sync.dma_start(hown0[tl * P : (tl + 1) * P, :], hsb[:])

            # ---- layer-0 table: full h0 = X @ W0' computed locally ----
            # global tile (r, tl): lo rows [r*LO + tl*128 ...] (tl < SPLIT_T)
            #                      hi rows [r*HI + tl*128 - LO ...]
            for r in range(R):
                for tl in range(T):
                    gcol = r * NPAD + tl * P
                    ph = psD.tile([P, 512], F32, tag="ph")
                    ph_hi = psE.tile([P, ROWG - 512], F32, tag="phh")
                    xsl = sb.tile([P, P], BF, tag="xsl")
                    nc.sync.dma_start(xsl[:], xfull_d[:, gcol : gcol + P])
                    nc.tensor.matmul(ph[:], lhsT=xsl[:],
                                     rhs=w0_sb[:, 0, 0:512],
                                     start=True, stop=True)
                    nc.tensor.matmul(ph_hi[:], lhsT=xsl[:],
                                     rhs=w0_sb[:, 0, 512:ROWG],
                                     start=True, stop=True)
                    hsb = sb.tile([P, ROWG], BF, tag="hsb640")
                    nc.vector.tensor_tensor(hsb[:, 0:512], ph[:],
                                            b0_sb[:, 0:512], mybir.AluOpType.add)
                    nc.vector.tensor_tensor(hsb[:, 512:ROWG], ph_hi[:],
                                            b0_sb[:, 512:ROWG],
                                            mybir.AluOpType.add)
                    if tl < SPLIT_T:
                        row = r * LO + tl * P
                        nc.sync.dma_start(tlo[0][row : row + P, :], hsb[:])
                    else:
                        row = r * HI + tl * P - LO
                        nc.sync.dma_start(thi[0][row : row + P, :], hsb[:])

            # ---- fused aggregation (+ next-layer h_ext) loops ----
            for L in range(3):
                rowg = ROWG if L < 2 else ROWG2
                grow = GROW if L < 2 else ROWG2
                nH = HEADS if L < 2 else 1
                ncols = HC if L < 2 else NCLS
                alow = ncols
                adoff = ncols + nH
                gtag = "g" if L < 2 else "g2"
                cls = "" if L < 2 else "2"
                if L == 0:
                    W_nx, b_nx, rowg_nx, KC = w1_sb, b1_sb, ROWG, 4
                elif L == 1:
                    W_nx, b_nx, rowg_nx, KC = w2_sb, b2_sb, ROWG2, 4
                else:
                    W_nx = None

                for t in range(T):
                    nlo, nhi = n16_lo[t], n16_hi[t]
                    blo, bhi = blo_l[t], bhi_l[t]
                    bt = bt_l[t]
                    bo = boff[t]

                    ilo = sb.tile([P, ILOMAX], I16, tag="ilo")
                    ihi = sb.tile([P, IHIMAX], I16, tag="ihi")
                    nc.sync.dma_start(ilo[:, 0 : nlo // 16],
                                      ilo_d[:, iolo[t] : iolo[t + 1]])
                    nc.sync.dma_start(ihi[:, 0 : nhi // 16],
                                      ihi_d[:, iohi[t] : iohi[t + 1]])
                    S_sb = sb.tile([P, BMAX * P], BF, tag="S")
                    St_sb = sb.tile([P, BMAX * P], BF, tag="St")
                    nc.sync.dma_start(S_sb[:, 0 : bt * P],
                                      smat_d[:, bo * P : (bo + bt) * P])
                    nc.sync.dma_start(St_sb[:, 0 : bt * P],
                                      stmat_d[:, bo * P : (bo + bt) * P])
                    # own rows for self-loop + al_d
                    loc = sb.tile([P, rowg], BF, tag=f"loc{cls}")
                    if L == 0:
                        nc.sync.dma_start(loc[:], hown0[t * P : (t + 1) * P, :])
                    elif t < SPLIT_T:
                        nc.sync.dma_start(loc[:], hlo[L][t * P : (t + 1) * P, :])
                    else:
                        r0 = t * P - LO
                        nc.sync.dma_start(loc[:], hhi[L][r0 : r0 + P, :])

                    g = sbg.tile([P, BMAX, grow], BF, tag=gtag)
                    nc.gpsimd.dma_gather(
                        g[:, 0:blo, :], tlo[L][:, :], ilo[:, 0 : nlo // 16],
                        num_idxs=nlo, num_idxs_reg=nlo, elem_size=grow)
                    nc.gpsimd.dma_gather(
                        g[:, blo:bt, :], thi[L][:, :], ihi[:, 0 : nhi // 16],
                        num_idxs=nhi, num_idxs_reg=nhi, elem_size=grow)

                    psmall = psB.tile([P, P], F32, tag="psmall")
                    pad_ps = psmall[:, 0 : bt * nH]
                    pd = psmall[:, 96 : 96 + nH]
                    for b in range(bt):
                        nc.tensor.matmul(
                            pad_ps[:, b * nH : (b + 1) * nH],
                            lhsT=St_sb[:, b * P : (b + 1) * P],
                            rhs=loc[:, adoff : adoff + nH],
                            start=True, stop=True)

                    logits = sb.tile([P, BMAX * nH], F32, tag=f"logits{cls}")
                    nc.vector.tensor_tensor(
                        logits[:, 0 : bt * nH].rearrange("p (b h) -> p b h", b=bt),
                        g[:, 0:bt, alow : alow + nH],
                        pad_ps[:].rearrange("p (b h) -> p b h", b=bt),
                        mybir.AluOpType.add)
                    lr = sb.tile([P, BMAX * nH], F32, tag=f"lr{cls}")
                    nc.vector.tensor_scalar_mul(lr[:, 0 : bt * nH],
                                                logits[:, 0 : bt * nH], NEG)
                    nc.vector.tensor_tensor(lr[:, 0 : bt * nH], lr[:, 0 : bt * nH],
                                            logits[:, 0 : bt * nH],
                                            mybir.AluOpType.max)
                    w = sb.tile([P, BMAX * nH], BF, tag=f"w{cls}")
                    nc.scalar.activation(w[:, 0 : bt * nH], lr[:, 0 : bt * nH],
                                         mybir.ActivationFunctionType.Exp)

                    sl = sb.tile([P, 2 * nH], F32, tag=f"sl{cls}")
                    nc.vector.tensor_tensor(sl[:, 0:nH],
                                            loc[:, alow : alow + nH],
                                            loc[:, adoff : adoff + nH],
                                            mybir.AluOpType.add)
                    nc.vector.tensor_scalar_mul(sl[:, nH : 2 * nH], sl[:, 0:nH], NEG)
                    nc.vector.tensor_tensor(sl[:, nH : 2 * nH], sl[:, nH : 2 * nH],
                                            sl[:, 0:nH], mybir.AluOpType.max)
                    ws = sb.tile([P, nH], F32, tag=f"ws{cls}")
                    nc.scalar.activation(ws[:], sl[:, nH : 2 * nH],
                                         mybir.ActivationFunctionType.Exp)

                    nc.vector.tensor_tensor(
                        g[:, 0:bt, 0:ncols].rearrange("p b (h c) -> p b h c", h=nH),
                        g[:, 0:bt, 0:ncols].rearrange("p b (h c) -> p b h c", h=nH),
                        w[:, 0 : bt * nH].rearrange("p (b h) -> p b h", b=bt)
                            .unsqueeze(3)
                            .to_broadcast([P, bt, nH, ncols // nH]),
                        mybir.AluOpType.mult)

                    po = psA.tile([P, HC], F32, tag="po")
                    for b in range(bt):
                        nc.tensor.matmul(po[:, 0:ncols],
                                         lhsT=S_sb[:, b * P : (b + 1) * P],
                                         rhs=g[:, b, 0:ncols],
                                         start=(b == 0), stop=(b == bt - 1))
                        nc.tensor.matmul(pd[:],
                                         lhsT=S_sb[:, b * P : (b + 1) * P],
                                         rhs=w[:, b * nH : (b + 1) * nH],
                                         start=(b == 0), stop=(b == bt - 1))

                    den = sb.tile([P, nH], F32, tag=f"den{cls}")
                    nc.vector.tensor_tensor(den[:], pd[:], ws[:],
                                            mybir.AluOpType.add)
                    rden = sb.tile([P, nH], F32, tag=f"rden{cls}")
                    nc.vector.reciprocal(rden[:], den[:])
                    wr = sb.tile([P, nH], F32, tag=f"wr{cls}")
                    nc.vector.tensor_tensor(wr[:], ws[:], rden[:],
                                            mybir.AluOpType.mult)

                    xn = sb.tile([P, ncols], F32, tag=f"xn{cls}")
                    nc.vector.tensor_tensor(
                        xn[:].rearrange("p (h c) -> p h c", h=nH),
                        po[:, 0:ncols].rearrange("p (h c) -> p h c", h=nH),
                        rden[:].unsqueeze(2).to_broadcast([P, nH, ncols // nH]),
                        mybir.AluOpType.mult)
                    t2 = sb.tile([P, ncols], F32, tag=f"t2{cls}")
                    nc.vector.tensor_tensor(
                        t2[:].rearrange("p (h c) -> p h c", h=nH),
                        loc[:, 0:ncols].rearrange("p (h c) -> p h c", h=nH),
                        wr[:].unsqueeze(2).to_broadcast([P, nH, ncols // nH]),
                        mybir.AluOpType.mult)
                    nc.vector.tensor_tensor(xn[:], xn[:], t2[:],
                                            mybir.AluOpType.add)

                    if L < 2:
                        m = sb.tile([P, ncols], F32, tag="elum")
                        nc.scalar.activation(m[:], xn[:],
                                             mybir.ActivationFunctionType.Relu,
                                             scale=-1.0)
                        em = sb.tile([P, ncols], F32, tag="eluem")
                        nc.scalar.activation(em[:], m[:],
                                             mybir.ActivationFunctionType.Exp,
                                             scale=-1.0)
                        xe = sb.tile([P, ncols], F32, tag="xe")
                        nc.scalar.activation(xe[:], xn[:],
                                             mybir.ActivationFunctionType.Relu)
                        nc.vector.tensor_tensor(xe[:], xe[:], em[:],
                                                mybir.AluOpType.add)
                        XtT = sb.tile([P, 4, P], BF, tag="XtT")
                        for c4 in range(4):
                            pt = psC.tile([P, P], F32, tag="pt")
                            nc.tensor.transpose(
                                pt[:], xe[:, c4 * P : (c4 + 1) * P], ident[:])
                            nc.scalar.copy(XtT[:, c4, :], pt[:])
                        ph = psD.tile([P, 512], F32, tag="ph")
                        n1 = min(512, rowg_nx)
                        for kc in range(KC):
                            nc.tensor.matmul(ph[:, 0:n1],
                                             lhsT=XtT[:, kc, :],
                                             rhs=W_nx[:, kc, 0:n1],
                                             start=(kc == 0), stop=(kc == KC - 1))
                        hsb = sb.tile([P, rowg_nx], BF, tag=f"hsb{rowg_nx}")
                        if rowg_nx > 512:
                            ph_hi = psE.tile([P, rowg_nx - 512], F32, tag="phh")
                            for kc in range(KC):
                                nc.tensor.matmul(ph_hi[:],
                                                 lhsT=XtT[:, kc, :],
                                                 rhs=W_nx[:, kc, 512:rowg_nx],
                                                 start=(kc == 0), stop=(kc == KC - 1))
                            nc.vector.tensor_tensor(hsb[:, 512:rowg_nx], ph_hi[:],
                                                    b_nx[:, 512:rowg_nx],
                                                    mybir.AluOpType.add)
                        nc.vector.tensor_tensor(hsb[:, 0:n1], ph[:, 0:n1],
                                                b_nx[:, 0:n1],
                                                mybir.AluOpType.add)
                        hext_store(L + 1, t, hsb)
                    else:
                        rows = min(P, PER - t * P)
                        nc.sync.dma_start(out_d[t * P : t * P + rows, :],
                                          xn[:rows, 0:NCLS])

    nc.compile()
    nc.m = get_hw_module(nc.m)
    return nc


_CACHE = {}


def _get_nc(cfg, meta):
    key = (tuple(sorted(cfg.items())),
           tuple(meta["n16_lo"]), tuple(meta["n16_hi"]))
    if key not in _CACHE:
        _CACHE[key] = build_gat_nc(cfg, meta)
    return _CACHE[key]


def run(inputs, trace=False):
    cfg = real_cfg()
    in_maps, meta = host_prepare(inputs, cfg)
    nc = _get_nc(cfg, meta)
    res = bass_utils.run_bass_kernel_spmd(
        nc, in_maps, core_ids=list(range(cfg["R"])), trace=trace)
    out = np.concatenate([res.results[r]["out"] for r in range(cfg["R"])], axis=0)
    return out[: cfg["N"]], res


def kernel(**inputs) -> np.ndarray:
    out, _ = run(inputs, trace=False)
    return out.astype(np.float32)


# revision 10
# speedup vs baseline: 1.0020x; 1.0020x over previous
"""GAT (3-layer, PyG-style) Trainium2 Bass kernel, sharded across 8 NeuronCores.

v3: dst-range graph-parallel sharding, fused per-tile pipeline.
- Layer-0's gather table h0 = X @ W0' is computed redundantly per core from
  the full (replicated) X, so layer 0 needs no collectives at all.
- Layers 1/2 exchange their 528-col h_ext tables via AllGathers chunked 5
  ways with strided output views, streamed during the previous layer's
  aggregation so only the last chunk's wire time is exposed.
- Per dst tile: dma_gather of in-edge source rows (lo/hi int16 split,
  per-tile static trimmed num_idxs), edge weights w = exp(lrelu(als+ald))
  with al_d per edge via host-precomputed S^T selection-matrix matmuls,
  numerator/denominator scatter-adds as S^T matmuls in PSUM, and the next
  layer's h_ext matmul fused in the same loop. ELU's "-1" is folded into
  the next layer's bias. Self-loops are analytic.

kernel(**inputs) takes FULL inputs, returns the FULL [N, 16] output.
"""

import sys

sys.path.insert(0, "/opt/trn_rl_repo")

import numpy as np

import concourse.bass as bass
import concourse.mybir as mybir
import concourse.tile as tile
from concourse import bacc
from concourse import bass_utils
from concourse.bass_interp import get_hw_module
from concourse.masks import make_identity
from concourse import library_config

F32 = mybir.dt.float32
BF = mybir.dt.bfloat16
I16 = mybir.dt.int16
import ml_dtypes
NPBF = ml_dtypes.bfloat16
P = 128


def real_cfg():
    R = 8
    N = 50000
    PER = N // R                      # 6250 nodes per core
    T = (PER + P - 1) // P            # 49 dst tiles per core
    return dict(
        R=R, N=N, PER=PER, T=T, NPAD=T * P,
        F_IN=128, HID=64, HEADS=8, N_CLASSES=16,
        NEG=0.2, SPLIT_T=31,
    )


ROWG = 640     # h_ext row width, layers 0/1: [512 feat | 8 al_s | 8 al_d | pad]
GROW = 640     # gather row width (256B multiple)
ROWG2 = 128    # layer-2 table: [16 out | 1 al_s | 1 al_d | pad]


# ---------------------------------------------------------------------------
# Host-side preprocessing
# ---------------------------------------------------------------------------

def _wrap16(flat):
    n = flat.shape[-1]
    w = flat.reshape(n // 16, 16).T
    return np.ascontiguousarray(np.tile(w, (8, 1)), np.int16)


def host_prepare(inputs, cfg):
    R, N, PER, T, NPAD = cfg["R"], cfg["N"], cfg["PER"], cfg["T"], cfg["NPAD"]
    F_IN, HID, HEADS, NCLS = cfg["F_IN"], cfg["HID"], cfg["HEADS"], cfg["N_CLASSES"]
    HC = HID * HEADS
    SPLIT_T = cfg["SPLIT_T"]
    LO = SPLIT_T * P
    HI = NPAD - LO

    x = np.asarray(inputs["x"], np.float32)
    ei = np.asarray(inputs["edge_index"])
    src = ei[0].astype(np.int64)
    dst = ei[1].astype(np.int64)

    core = dst // PER
    dloc = (dst - core * PER).astype(np.int64)
    sloc = (src % PER).astype(np.int64)
    srank = (src // PER).astype(np.int64)
    is_lo = sloc < LO
    tile_of = dloc // P

    cl = np.zeros((R, T), np.int64)
    ch = np.zeros((R, T), np.int64)
    np.add.at(cl, (core[is_lo], tile_of[is_lo]), 1)
    np.add.at(ch, (core[~is_lo], tile_of[~is_lo]), 1)
    n16_lo = ((cl.max(axis=0) + 15) // 16 * 16).astype(np.int64)
    n16_hi = ((ch.max(axis=0) + 15) // 16 * 16).astype(np.int64)
    blo = (n16_lo + P - 1) // P
    bhi = (n16_hi + P - 1) // P
    bt = blo + bhi
    boff = np.concatenate([[0], np.cumsum(bt)])
    TOTB = int(boff[-1])
    iolo = np.concatenate([[0], np.cumsum(n16_lo // 16)])
    iohi = np.concatenate([[0], np.cumsum(n16_hi // 16)])
    ILO_TOT = int(iolo[-1])
    IHI_TOT = int(iohi[-1])

    grow = np.where(is_lo, srank * LO + sloc, srank * HI + (sloc - LO))

    order = np.lexsort(((~is_lo).astype(np.int64), tile_of, core))
    g_s = grow[order]
    d_s = dloc[order]
    core_s = core[order]
    tile_s = tile_of[order]
    lo_s = is_lo[order]

    grp = core_s * (2 * T) + tile_s * 2 + (~lo_s).astype(np.int64)
    grp_start = np.searchsorted(grp, np.arange(R * T * 2), side="left")
    pos = np.arange(len(grp)) - grp_start[grp]

    ilo_all = np.zeros((R, ILO_TOT * 16), np.int16)
    ihi_all = np.zeros((R, IHI_TOT * 16), np.int16)
    lo_m = lo_s
    hi_m = ~lo_s
    ilo_all[core_s[lo_m], iolo[tile_s[lo_m]] * 16 + pos[lo_m]] = g_s[lo_m].astype(np.int16)
    ihi_all[core_s[hi_m], iohi[tile_s[hi_m]] * 16 + pos[hi_m]] = g_s[hi_m].astype(np.int16)

    S_all = np.zeros((R, P, TOTB * P), NPBF)
    St_all = np.zeros((R, P, TOTB * P), NPBF)
    slot = np.where(lo_m, pos, blo[tile_s] * P + pos)
    blk = boff[tile_s] + slot // P
    prt = slot % P
    dt_s = d_s - tile_s * P
    S_all[core_s, prt, blk * P + dt_s] = np.float32(1)
    St_all[core_s, dt_s, blk * P + prt] = np.float32(1)

    def wext(W, a_s, a_d, ncols):
        Fin = W.shape[0]
        H, C = a_s.shape
        Wr = W.reshape(Fin, H, C)
        We = np.zeros((Fin, ncols), np.float32)
        We[:, : H * C] = W
        We[:, H * C : H * C + H] = np.einsum("fhc,hc->fh", Wr, a_s)
        We[:, H * C + H : H * C + 2 * H] = np.einsum("fhc,hc->fh", Wr, a_d)
        return We

    W0e = wext(np.asarray(inputs["W0"], np.float32),
               np.asarray(inputs["a_s0"], np.float32),
               np.asarray(inputs["a_d0"], np.float32), ROWG)
    W1e = wext(np.asarray(inputs["W1"], np.float32),
               np.asarray(inputs["a_s1"], np.float32),
               np.asarray(inputs["a_d1"], np.float32), ROWG)
    W2e = wext(np.asarray(inputs["W2"], np.float32),
               np.asarray(inputs["a_s2"], np.float32),
               np.asarray(inputs["a_d2"], np.float32), ROWG2)

    def bext(b, ncols, Wfull):
        be = np.zeros((1, ncols), np.float32)
        be[0, : b.shape[0]] = b
        if Wfull is not None:
            be[0, :] -= Wfull.sum(axis=0)   # fold ELU's -1 into the bias
        return np.ascontiguousarray(np.broadcast_to(be, (P, ncols)))

    b0e = bext(np.asarray(inputs["b0"], np.float32), ROWG, None)
    b1e = bext(np.asarray(inputs["b1"], np.float32), ROWG, W1e)
    b2e = bext(np.asarray(inputs["b2"], np.float32), ROWG2, W2e)

    W0b = W0e.astype(NPBF)
    W1b = np.ascontiguousarray(
        W1e.reshape(4, P, ROWG).transpose(1, 0, 2)).astype(NPBF)
    W2b = np.ascontiguousarray(
        W2e.reshape(4, P, ROWG2).transpose(1, 0, 2)).astype(NPBF)

    # full node-feature matrix, feature-major, padded per-rank to NPAD
    xf = np.zeros((F_IN, R * NPAD), np.float32)
    for r in range(R):
        xf[:, r * NPAD : r * NPAD + PER] = x[r * PER : (r + 1) * PER].T
    xf = xf.astype(NPBF)

    in_maps = []
    for r in range(R):
        lo_w = np.zeros((P, ILO_TOT), np.int16)
        hi_w = np.zeros((P, IHI_TOT), np.int16)
        for t in range(T):
            lo_w[:, iolo[t] : iolo[t + 1]] = _wrap16(
                ilo_all[r, iolo[t] * 16 : iolo[t + 1] * 16])
            hi_w[:, iohi[t] : iohi[t + 1]] = _wrap16(
                ihi_all[r, iohi[t] * 16 : iohi[t + 1] * 16])
        xt = np.zeros((F_IN, NPAD), np.float32)
        xt[:, :PER] = x[r * PER : (r + 1) * PER].T
        in_maps.append({
            "xt0": xt.astype(NPBF),
            "xfull": xf,
            "w0e": W0b, "w1e": W1b, "w2e": W2b,
            "b0r": b0e[0:1, :].astype(NPBF),
            "b1e": b1e, "b2e": b2e,
            "ilo": lo_w, "ihi": hi_w,
            "smat": S_all[r], "stmat": St_all[r],
        })

    meta = dict(
        n16_lo=n16_lo.tolist(), n16_hi=n16_hi.tolist(),
        blo=blo.tolist(), bhi=bhi.tolist(), bt=bt.tolist(),
        boff=boff.tolist(), iolo=iolo.tolist(), iohi=iohi.tolist(),
        TOTB=TOTB, ILO_TOT=ILO_TOT, IHI_TOT=IHI_TOT,
        BMAX=int(bt.max()),
    )
    return in_maps, meta


# ---------------------------------------------------------------------------
# Device program
# ---------------------------------------------------------------------------

def build_gat_nc(cfg, meta):
    R, PER, T, NPAD = cfg["R"], cfg["PER"], cfg["T"], cfg["NPAD"]
    F_IN, HID, HEADS, NCLS = cfg["F_IN"], cfg["HID"], cfg["HEADS"], cfg["N_CLASSES"]
    NEG = cfg["NEG"]
    HC = HID * HEADS
    SPLIT_T = cfg["SPLIT_T"]
    LO = SPLIT_T * P
    HI = NPAD - LO
    n16_lo, n16_hi = meta["n16_lo"], meta["n16_hi"]
    blo_l, bhi_l, bt_l = meta["blo"], meta["bhi"], meta["bt"]
    boff, iolo, iohi = meta["boff"], meta["iolo"], meta["iohi"]
    TOTB, ILO_TOT, IHI_TOT = meta["TOTB"], meta["ILO_TOT"], meta["IHI_TOT"]
    BMAX = meta["BMAX"]
    ILOMAX = max(n16_lo) // 16
    IHIMAX = max(n16_hi) // 16

    nc = bacc.Bacc("TRN2", target_bir_lowering=False, debug=False,
                   num_devices=R)

    xfull_d = nc.dram_tensor("xfull", [F_IN, R * NPAD], BF, kind="ExternalInput")
    xt0_d = nc.dram_tensor("xt0", [F_IN, NPAD], BF, kind="ExternalInput")
    w0e_d = nc.dram_tensor("w0e", [P, ROWG], BF, kind="ExternalInput")
    w1e_d = nc.dram_tensor("w1e", [P, 4, ROWG], BF, kind="ExternalInput")
    w2e_d = nc.dram_tensor("w2e", [P, 4, ROWG2], BF, kind="ExternalInput")
    b0r_d = nc.dram_tensor("b0r", [1, ROWG], BF, kind="ExternalInput")
    b1e_d = nc.dram_tensor("b1e", [P, ROWG], F32, kind="ExternalInput")
    b2e_d = nc.dram_tensor("b2e", [P, ROWG2], F32, kind="ExternalInput")
    ilo_d = nc.dram_tensor("ilo", [P, ILO_TOT], I16, kind="ExternalInput")
    ihi_d = nc.dram_tensor("ihi", [P, IHI_TOT], I16, kind="ExternalInput")
    smat_d = nc.dram_tensor("smat", [P, TOTB * P], BF, kind="ExternalInput")
    stmat_d = nc.dram_tensor("stmat", [P, TOTB * P], BF, kind="ExternalInput")
    out_d = nc.dram_tensor("out", [PER, NCLS], F32, kind="ExternalOutput")

    rg = [list(range(R))]

    with tile.TileContext(nc) as tc:
        with (
            tc.tile_pool(name="pers", bufs=1) as pers,
            tc.tile_pool(name="sb", bufs=3) as sb,
            tc.tile_pool(name="sbg", bufs=4) as sbg,
            tc.tile_pool(name="psA", bufs=2, space="PSUM") as psA,
            tc.tile_pool(name="psB", bufs=2, space="PSUM") as psB,
            tc.tile_pool(name="psC", bufs=1, space="PSUM") as psC,
            tc.tile_pool(name="psD", bufs=2, space="PSUM") as psD,
            tc.tile_pool(name="psE", bufs=1, space="PSUM") as psE,
            tc.tile_pool(name="dram", bufs=1, space="DRAM") as dram,
        ):
            nc.gpsimd.load_library(library_config.mlp)

            ident = pers.tile([P, P], F32)
            make_identity(nc, ident[:])
            w0_sb = pers.tile([P, 1, ROWG], BF)
            w1_sb = pers.tile([P, 4, ROWG], BF)
            w2_sb = pers.tile([P, 4, ROWG2], BF)
            b0r_sb = pers.tile([1, ROWG], BF)
            ones1 = pers.tile([1, P], BF)
            nc.vector.memset(ones1[:], 1.0)
            b1_sb = pers.tile([P, ROWG], F32)
            b2_sb = pers.tile([P, ROWG2], F32)
            nc.sync.dma_start(w0_sb[:, 0, :], w0e_d[:, :])
            nc.sync.dma_start(w1_sb[:], w1e_d[:, :, :])
            nc.sync.dma_start(w2_sb[:], w2e_d[:, :, :])
            nc.sync.dma_start(b0r_sb[:], b0r_d[:, :])
            nc.sync.dma_start(b1_sb[:], b1e_d[:, :])
            nc.sync.dma_start(b2_sb[:], b2e_d[:, :])

            for _ in range(4):
                gg = sbg.tile([P, BMAX, GROW], BF, tag="g")
                nc.vector.memset(gg[:].rearrange("p a b -> p (a b)"), 0.0)
                gg2 = sbg.tile([P, BMAX, ROWG2], BF, tag="g2")
                nc.vector.memset(gg2[:].rearrange("p a b -> p (a b)"), 0.0)

            # gather tables (row width GROW; cols ROWG..GROW stay garbage)
            tlo = [dram.tile([R * LO, GROW], BF, name="tlo0"),
                   dram.tile([R * LO, GROW], BF, addr_space="Shared", name="tlo1"),
                   dram.tile([R * LO, ROWG2], BF, addr_space="Shared", name="tlo2")]
            thi = [dram.tile([R * HI, GROW], BF, name="thi0"),
                   dram.tile([R * HI, GROW], BF, addr_space="Shared", name="thi1"),
                   dram.tile([R * HI, ROWG2], BF, addr_space="Shared", name="thi2")]
            # local h_ext staging (input of the chunked AllGathers), L=1,2
            hlo = [None,
                   dram.tile([LO, ROWG], BF, name="hlo1"),
                   dram.tile([LO, ROWG2], BF, name="hlo2")]
            hhi = [None,
                   dram.tile([HI, ROWG], BF, name="hhi1"),
                   dram.tile([HI, ROWG2], BF, name="hhi2")]
            hown0 = dram.tile([NPAD, ROWG], BF, name="hown0")

            def ag_chunks(L, t, rowg_t):
                """Fire the layer-L table AllGathers (one per group)."""
                if t == SPLIT_T - 1:
                    nc.gpsimd.collective_compute(
                        "AllGather", mybir.AluOpType.bypass,
                        replica_groups=rg,
                        ins=[hlo[L][:, :]], outs=[tlo[L][:, :]])
                if t == T - 1:
                    nc.gpsimd.collective_compute(
                        "AllGather", mybir.AluOpType.bypass,
                        replica_groups=rg,
                        ins=[hhi[L][:, :]], outs=[thi[L][:, :]])

            def hext_store(L, t, hsb):
                """Store one tile's h_ext rows for layer L (L>=1) + chunks."""
                if t < SPLIT_T:
                    nc.sync.dma_start(hlo[L][t * P : (t + 1) * P, :], hsb[:])
                else:
                    r0 = t * P - LO
                    nc.sync.dma_start(hhi[L][r0 : r0 + P, :], hsb[:])
                ag_chunks(L, t, None)

            # ---- layer-0 own rows -> hown0 (rank-independent layout) ----
            for tl in range(T):
                ph = psD.tile([P, 512], F32, tag="ph")
                ph_hi = psE.tile([P, ROWG - 512], F32, tag="phh")
                xsl = sb.tile([P, P], BF, tag="xsl")
                nc.sync.dma_start(xsl[:], xt0_d[:, tl * P : (tl + 1) * P])
                nc.tensor.matmul(ph[:], lhsT=xsl[:], rhs=w0_sb[:, 0, 0:512],
                                 start=True, stop=False)
                nc.tensor.matmul(ph[:], lhsT=ones1[:], rhs=b0r_sb[:, 0:512],
                                 start=False, stop=True)
                nc.tensor.matmul(ph_hi[:], lhsT=xsl[:], rhs=w0_sb[:, 0, 512:ROWG],
                                 start=True, stop=False)
                nc.tensor.matmul(ph_hi[:], lhsT=ones1[:], rhs=b0r_sb[:, 512:ROWG],
                                 start=False, stop=True)
                hsb = sb.tile([P, ROWG], BF, tag="hsb640")
                nc.scalar.copy(hsb[:, 0:512], ph[:])
                nc.scalar.copy(hsb[:, 512:ROWG], ph_hi[:])
                nc.sync.dma_start(hown0[tl * P : (tl + 1) * P, :], hsb[:])

            # ---- layer-0 table: full h0 = X @ W0' computed locally ----
            # global tile (r, tl): lo rows [r*LO + tl*128 ...] (tl < SPLIT_T)
            #                      hi rows [r*HI + tl*128 - LO ...]
            for r in range(R):
                for tl in range(T):
                    gcol = r * NPAD + tl * P
                    ph = psD.tile([P, 512], F32, tag="ph")
                    ph_hi = psE.tile([P, ROWG - 512], F32, tag="phh")
                    xsl = sb.tile([P, P], BF, tag="xsl")
                    nc.sync.dma_start(xsl[:], xfull_d[:, gcol : gcol + P])
                    nc.tensor.matmul(ph[:], lhsT=xsl[:],
                                     rhs=w0_sb[:, 0, 0:512],
                                     start=True, stop=False)
                    nc.tensor.matmul(ph[:], lhsT=ones1[:], rhs=b0r_sb[:, 0:512],
                                     start=False, stop=True)
                    nc.tensor.matmul(ph_hi[:], lhsT=xsl[:],
                                     rhs=w0_sb[:, 0, 512:ROWG],
                                     start=True, stop=False)
                    nc.tensor.matmul(ph_hi[:], lhsT=ones1[:],
                                     rhs=b0r_sb[:, 512:ROWG],
                                     start=False, stop=True)
                    hsb = sb.tile([P, ROWG], BF, tag="hsb640")
                    nc.scalar.copy(hsb[:, 0:512], ph[:])
                    nc.scalar.copy(hsb[:, 512:ROWG], ph_hi[:])
                    if tl < SPLIT_T:
                        row = r * LO + tl * P
                        nc.sync.dma_start(tlo[0][row : row + P, :], hsb[:])
                    else:
                        row = r * HI + tl * P - LO
                        nc.sync.dma_start(thi[0][row : row + P, :], hsb[:])

            # ---- fused aggregation (+ next-layer h_ext) loops ----
            for L in range(3):
                rowg = ROWG if L < 2 else ROWG2
                grow = GROW if L < 2 else ROWG2
                nH = HEADS if L < 2 else 1
                ncols = HC if L < 2 else NCLS
                alow = ncols
                adoff = ncols + nH
                gtag = "g" if L < 2 else "g2"
                cls = "" if L < 2 else "2"
                if L == 0:
                    W_nx, b_nx, rowg_nx, KC = w1_sb, b1_sb, ROWG, 4
                elif L == 1:
                    W_nx, b_nx, rowg_nx, KC = w2_sb, b2_sb, ROWG2, 4
                else:
                    W_nx = None

                for t in range(T):
                    nlo, nhi = n16_lo[t], n16_hi[t]
                    blo, bhi = blo_l[t], bhi_l[t]
                    bt = bt_l[t]
                    bo = boff[t]

                    ilo = sb.tile([P, ILOMAX], I16, tag="ilo")
                    ihi = sb.tile([P, IHIMAX], I16, tag="ihi")
                    nc.sync.dma_start(ilo[:, 0 : nlo // 16],
                                      ilo_d[:, iolo[t] : iolo[t + 1]])
                    nc.sync.dma_start(ihi[:, 0 : nhi // 16],
                                      ihi_d[:, iohi[t] : iohi[t + 1]])
                    S_sb = sb.tile([P, BMAX * P], BF, tag="S")
                    St_sb = sb.tile([P, BMAX * P], BF, tag="St")
                    nc.sync.dma_start(S_sb[:, 0 : bt * P],
                                      smat_d[:, bo * P : (bo + bt) * P])
                    nc.sync.dma_start(St_sb[:, 0 : bt * P],
                                      stmat_d[:, bo * P : (bo + bt) * P])
                    # own rows for self-loop + al_d
                    loc = sb.tile([P, rowg], BF, tag=f"loc{cls}")
                    if L == 0:
                        nc.sync.dma_start(loc[:], hown0[t * P : (t + 1) * P, :])
                    elif t < SPLIT_T:
                        nc.sync.dma_start(loc[:], hlo[L][t * P : (t + 1) * P, :])
                    else:
                        r0 = t * P - LO
                        nc.sync.dma_start(loc[:], hhi[L][r0 : r0 + P, :])

                    g = sbg.tile([P, BMAX, grow], BF, tag=gtag)
                    nc.gpsimd.dma_gather(
                        g[:, 0:blo, :], tlo[L][:, :], ilo[:, 0 : nlo // 16],
                        num_idxs=nlo, num_idxs_reg=nlo, elem_size=grow)
                    nc.gpsimd.dma_gather(
                        g[:, blo:bt, :], thi[L][:, :], ihi[:, 0 : nhi // 16],
                        num_idxs=nhi, num_idxs_reg=nhi, elem_size=grow)

                    psmall = psB.tile([P, P], F32, tag="psmall")
                    pad_ps = psmall[:, 0 : bt * nH]
                    pd = psmall[:, 96 : 96 + nH]
                    for b in range(bt):
                        nc.tensor.matmul(
                            pad_ps[:, b * nH : (b + 1) * nH],
                            lhsT=St_sb[:, b * P : (b + 1) * P],
                            rhs=loc[:, adoff : adoff + nH],
                            start=True, stop=True)

                    logits = sb.tile([P, BMAX * nH], F32, tag=f"logits{cls}")
                    nc.vector.tensor_tensor(
                        logits[:, 0 : bt * nH].rearrange("p (b h) -> p b h", b=bt),
                        g[:, 0:bt, alow : alow + nH],
                        pad_ps[:].rearrange("p (b h) -> p b h", b=bt),
                        mybir.AluOpType.add)
                    lr = sb.tile([P, BMAX * nH], F32, tag=f"lr{cls}")
                    nc.vector.tensor_scalar_mul(lr[:, 0 : bt * nH],
                                                logits[:, 0 : bt * nH], NEG)
                    nc.vector.tensor_tensor(lr[:, 0 : bt * nH], lr[:, 0 : bt * nH],
                                            logits[:, 0 : bt * nH],
                                            mybir.AluOpType.max)
                    w = sb.tile([P, BMAX * nH], BF, tag=f"w{cls}")
                    nc.scalar.activation(w[:, 0 : bt * nH], lr[:, 0 : bt * nH],
                                         mybir.ActivationFunctionType.Exp)

                    sl = sb.tile([P, 2 * nH], F32, tag=f"sl{cls}")
                    nc.vector.tensor_tensor(sl[:, 0:nH],
                                            loc[:, alow : alow + nH],
                                            loc[:, adoff : adoff + nH],
                                            mybir.AluOpType.add)
                    nc.vector.tensor_scalar_mul(sl[:, nH : 2 * nH], sl[:, 0:nH], NEG)
                    nc.vector.tensor_tensor(sl[:, nH : 2 * nH], sl[:, nH : 2 * nH],
                                            sl[:, 0:nH], mybir.AluOpType.max)
                    ws = sb.tile([P, nH], F32, tag=f"ws{cls}")
                    nc.scalar.activation(ws[:], sl[:, nH : 2 * nH],
                                         mybir.ActivationFunctionType.Exp)

                    nc.vector.tensor_tensor(
                        g[:, 0:bt, 0:ncols].rearrange("p b (h c) -> p b h c", h=nH),
                        g[:, 0:bt, 0:ncols].rearrange("p b (h c) -> p b h c", h=nH),
                        w[:, 0 : bt * nH].rearrange("p (b h) -> p b h", b=bt)
                            .unsqueeze(3)
                            .to_broadcast([P, bt, nH, ncols // nH]),
                        mybir.AluOpType.mult)

                    po = psA.tile([P, HC], F32, tag="po")
                    for b in range(bt):
                        nc.tensor.matmul(po[:, 0:ncols],
                                         lhsT=S_sb[:, b * P : (b + 1) * P],
                                         rhs=g[:, b, 0:ncols],
                                         start=(b == 0), stop=(b == bt - 1))
                        nc.tensor.matmul(pd[:],
                                         lhsT=S_sb[:, b * P : (b + 1) * P],
                                         rhs=w[:, b * nH : (b + 1) * nH],
                                         start=(b == 0), stop=(b == bt - 1))

                    den = sb.tile([P, nH], F32, tag=f"den{cls}")
                    nc.vector.tensor_tensor(den[:], pd[:], ws[:],
                                            mybir.AluOpType.add)
                    rden = sb.tile([P, nH], F32, tag=f"rden{cls}")
                    nc.vector.reciprocal(rden[:], den[:])
                    wr = sb.tile([P, nH], F32, tag=f"wr{cls}")
                    nc.vector.tensor_tensor(wr[:], ws[:], rden[:],
                                            mybir.AluOpType.mult)

                    xn = sb.tile([P, ncols], F32, tag=f"xn{cls}")
                    nc.vector.tensor_tensor(
                        xn[:].rearrange("p (h c) -> p h c", h=nH),
                        po[:, 0:ncols].rearrange("p (h c) -> p h c", h=nH),
                        rden[:].unsqueeze(2).to_broadcast([P, nH, ncols // nH]),
                        mybir.AluOpType.mult)
                    t2 = sb.tile([P, ncols], F32, tag=f"t2{cls}")
                    nc.vector.tensor_tensor(
                        t2[:].rearrange("p (h c) -> p h c", h=nH),
                        loc[:, 0:ncols].rearrange("p (h c) -> p h c", h=nH),
                        wr[:].unsqueeze(2).to_broadcast([P, nH, ncols // nH]),
                        mybir.AluOpType.mult)
                    nc.vector.tensor_tensor(xn[:], xn[:], t2[:],
                                            mybir.AluOpType.add)

                    if L < 2:
                        m = sb.tile([P, ncols], F32, tag="elum")
                        nc.scalar.activation(m[:], xn[:],
                                             mybir.ActivationFunctionType.Relu,
                                             scale=-1.0)
                        em = sb.tile([P, ncols], F32, tag="eluem")
                        nc.scalar.activation(em[:], m[:],
                                             mybir.ActivationFunctionType.Exp,
                                             scale=-1.0)
                        xe = sb.tile([P, ncols], F32, tag="xe")
                        nc.scalar.activation(xe[:], xn[:],
                                             mybir.ActivationFunctionType.Relu)
                        nc.vector.tensor_tensor(xe[:], xe[:], em[:],
                                                mybir.AluOpType.add)
                        XtT = sb.tile([P, 4, P], BF, tag="XtT")
                        for c4 in range(4):
                            pt = psC.tile([P, P], F32, tag="pt")
                            nc.tensor.transpose(
                                pt[:], xe[:, c4 * P : (c4 + 1) * P], ident[:])
                            nc.scalar.copy(XtT[:, c4, :], pt[:])
                        ph = psD.tile([P, 512], F32, tag="ph")
                        n1 = min(512, rowg_nx)
                        for kc in range(KC):
                            nc.tensor.matmul(ph[:, 0:n1],
                                             lhsT=XtT[:, kc, :],
                                             rhs=W_nx[:, kc, 0:n1],
                                             start=(kc == 0), stop=(kc == KC - 1))
                        hsb = sb.tile([P, rowg_nx], BF, tag=f"hsb{rowg_nx}")
                        if rowg_nx > 512:
                            ph_hi = psE.tile([P, rowg_nx - 512], F32, tag="phh")
                            for kc in range(KC):
                                nc.tensor.matmul(ph_hi[:],
                                                 lhsT=XtT[:, kc, :],
                                                 rhs=W_nx[:, kc, 512:rowg_nx],
                                                 start=(kc == 0), stop=(kc == KC - 1))
                            nc.vector.tensor_tensor(hsb[:, 512:rowg_nx], ph_hi[:],
                                                    b_nx[:, 512:rowg_nx],
                                                    mybir.AluOpType.add)
                        nc.vector.tensor_tensor(hsb[:, 0:n1], ph[:, 0:n1],
                                                b_nx[:, 0:n1],
                                                mybir.AluOpType.add)
                        hext_store(L + 1, t, hsb)
                    else:
                        rows = min(P, PER - t * P)
                        nc.sync.dma_start(out_d[t * P : t * P + rows, :],
                                          xn[:rows, 0:NCLS])

    nc.compile()
    nc.m = get_hw_module(nc.m)
    return nc


_CACHE = {}


def _get_nc(cfg, meta):
    key = (tuple(sorted(cfg.items())),
           tuple(meta["n16_lo"]), tuple(meta["n16_hi"]))
    if key not in _CACHE:
        _CACHE[key] = build_gat_nc(cfg, meta)
    return _CACHE[key]


def run(inputs, trace=False):
    cfg = real_cfg()
    in_maps, meta = host_prepare(inputs, cfg)
    nc = _get_nc(cfg, meta)
    res = bass_utils.run_bass_kernel_spmd(
        nc, in_maps, core_ids=list(range(cfg["R"])), trace=trace)
    out = np.concatenate([res.results[r]["out"] for r in range(cfg["R"])], axis=0)
    return out[: cfg["N"]], res


def kernel(**inputs) -> np.ndarray:
    out, _ = run(inputs, trace=False)
    return out.astype(np.float32)


# revision 12
# speedup vs baseline: 1.1116x; 1.1095x over previous
"""GAT (3-layer, PyG-style) Trainium2 Bass kernel, sharded across 8 NeuronCores.

v2: dst-range graph-parallel sharding with a fused per-tile pipeline.
Per layer, each dst tile gathers its in-edges' source rows from the
AllGathered h_ext table (dma_gather, lo/hi int16 split, per-tile trimmed
num_idxs), computes edge softmax weights w = exp(lrelu(al_s+al_d)) and
scatter-adds numerator/denominator via host-precomputed selection-matrix
matmuls (S, S^T shipped as inputs - no on-device is_equal builds). The
next layer's h_ext = elu(out) @ W' matmul is fused into the same tile
loop so PE work and the AllGather hide under the gather desc-gen, which
is the hard floor (~8ns/row on the Pool engine). ELU's "-1" is folded
into the next layer's bias on the host. Self-loops are analytic.

kernel(**inputs) takes FULL inputs, returns the FULL [N, 16] output.
"""

import sys

sys.path.insert(0, "/opt/trn_rl_repo")

import numpy as np

import concourse.bass as bass
import concourse.mybir as mybir
import concourse.tile as tile
from concourse import bacc
from concourse import bass_utils
from concourse.bass_interp import get_hw_module
from concourse.masks import make_identity
from concourse import library_config

F32 = mybir.dt.float32
BF = mybir.dt.bfloat16
I16 = mybir.dt.int16
import ml_dtypes
NPBF = ml_dtypes.bfloat16
P = 128


def real_cfg():
    R = 8
    N = 50000
    PER = N // R                      # 6250 nodes per core
    T = (PER + P - 1) // P            # 49 dst tiles per core
    return dict(
        R=R, N=N, PER=PER, T=T, NPAD=T * P,
        F_IN=128, HID=64, HEADS=8, N_CLASSES=16,
        NEG=0.2, SPLIT_T=31,
    )


# ---------------------------------------------------------------------------
# Host-side preprocessing
# ---------------------------------------------------------------------------

def _wrap16(flat):
    """int16 index list (len % 16 == 0) -> dma_gather idx layout [128, n/16]."""
    n = flat.shape[-1]
    w = flat.reshape(n // 16, 16).T                       # [16, n/16]
    return np.ascontiguousarray(np.tile(w, (8, 1)), np.int16)


def host_prepare(inputs, cfg):
    """Build per-core in_maps (numpy). Returns (in_maps, meta)."""
    R, N, PER, T, NPAD = cfg["R"], cfg["N"], cfg["PER"], cfg["T"], cfg["NPAD"]
    F_IN, HID, HEADS, NCLS = cfg["F_IN"], cfg["HID"], cfg["HEADS"], cfg["N_CLASSES"]
    HC = HID * HEADS
    SPLIT_T = cfg["SPLIT_T"]
    LO = SPLIT_T * P
    HI = NPAD - LO

    x = np.asarray(inputs["x"], np.float32)
    ei = np.asarray(inputs["edge_index"])
    src = ei[0].astype(np.int64)
    dst = ei[1].astype(np.int64)   # self-loops handled analytically on device

    core = dst // PER
    dloc = (dst - core * PER).astype(np.int64)
    sloc = (src % PER).astype(np.int64)
    srank = (src // PER).astype(np.int64)
    is_lo = sloc < LO
    tile_of = dloc // P

    # per (core, tile, group) counts
    cl = np.zeros((R, T), np.int64)
    ch = np.zeros((R, T), np.int64)
    np.add.at(cl, (core[is_lo], tile_of[is_lo]), 1)
    np.add.at(ch, (core[~is_lo], tile_of[~is_lo]), 1)
    # shared (max over cores) per-tile padded counts, 16-granular
    n16_lo = ((cl.max(axis=0) + 15) // 16 * 16).astype(np.int64)   # [T]
    n16_hi = ((ch.max(axis=0) + 15) // 16 * 16).astype(np.int64)
    blo = (n16_lo + P - 1) // P
    bhi = (n16_hi + P - 1) // P
    bt = blo + bhi
    boff = np.concatenate([[0], np.cumsum(bt)])       # [T+1]
    TOTB = int(boff[-1])
    iolo = np.concatenate([[0], np.cumsum(n16_lo // 16)])
    iohi = np.concatenate([[0], np.cumsum(n16_hi // 16)])
    ILO_TOT = int(iolo[-1])
    IHI_TOT = int(iohi[-1])

    # gather row ids within the lo / hi tables
    grow = np.where(is_lo, srank * LO + sloc, srank * HI + (sloc - LO))

    # order edges by (core, tile, group); position within group
    order = np.lexsort(((~is_lo).astype(np.int64), tile_of, core))
    g_s = grow[order]
    d_s = dloc[order]
    core_s = core[order]
    tile_s = tile_of[order]
    lo_s = is_lo[order]

    grp = core_s * (2 * T) + tile_s * 2 + (~lo_s).astype(np.int64)
    grp_start = np.searchsorted(grp, np.arange(R * T * 2), side="left")
    pos = np.arange(len(grp)) - grp_start[grp]

    # flat idx arrays per core (wrap16 layout), zero padded
    ilo_all = np.zeros((R, ILO_TOT * 16), np.int16)
    ihi_all = np.zeros((R, IHI_TOT * 16), np.int16)
    lo_m = lo_s
    hi_m = ~lo_s
    ilo_all[core_s[lo_m], iolo[tile_s[lo_m]] * 16 + pos[lo_m]] = g_s[lo_m].astype(np.int16)
    ihi_all[core_s[hi_m], iohi[tile_s[hi_m]] * 16 + pos[hi_m]] = g_s[hi_m].astype(np.int16)

    # S / St selection matrices, [128, TOTB*128] bf16 per core
    S_all = np.zeros((R, P, TOTB * P), NPBF)
    St_all = np.zeros((R, P, TOTB * P), NPBF)
    # slot within tile: lo slots first (blocks 0..blo-1), then hi
    slot = np.where(lo_m, pos, blo[tile_s] * P + pos)
    blk = boff[tile_s] + slot // P
    prt = slot % P
    dt_s = d_s - tile_s * P           # dst row within the tile (0..127)
    S_all[core_s, prt, blk * P + dt_s] = np.float32(1)
    St_all[core_s, dt_s, blk * P + prt] = np.float32(1)

    # weight assembly: W'[f, :] = [W | W.a_src | W.a_dst | pad]
    def wext(W, a_s, a_d, ncols):
        Fin = W.shape[0]
        H, C = a_s.shape
        Wr = W.reshape(Fin, H, C)
        We = np.zeros((Fin, ncols), np.float32)
        We[:, : H * C] = W
        We[:, H * C : H * C + H] = np.einsum("fhc,hc->fh", Wr, a_s)
        We[:, H * C + H : H * C + 2 * H] = np.einsum("fhc,hc->fh", Wr, a_d)
        return We

    ROWG = 640
    ROWG2 = 128
    W0e = wext(np.asarray(inputs["W0"], np.float32),
               np.asarray(inputs["a_s0"], np.float32),
               np.asarray(inputs["a_d0"], np.float32), ROWG)
    W1e = wext(np.asarray(inputs["W1"], np.float32),
               np.asarray(inputs["a_s1"], np.float32),
               np.asarray(inputs["a_d1"], np.float32), ROWG)
    W2e = wext(np.asarray(inputs["W2"], np.float32),
               np.asarray(inputs["a_s2"], np.float32),
               np.asarray(inputs["a_d2"], np.float32), ROWG2)

    def bext(b, ncols, Wfull):
        be = np.zeros((1, ncols), np.float32)
        be[0, : b.shape[0]] = b
        if Wfull is not None:
            # ELU output on device is elu(x)+1; fold the -1 into the bias.
            be[0, :] -= Wfull.sum(axis=0)
        return np.ascontiguousarray(np.broadcast_to(be, (P, ncols)))

    b0e = bext(np.asarray(inputs["b0"], np.float32), ROWG, None)
    b1e = bext(np.asarray(inputs["b1"], np.float32), ROWG, W1e)
    b2e = bext(np.asarray(inputs["b2"], np.float32), ROWG2, W2e)

    W0b = W0e.astype(NPBF)
    W1b = np.ascontiguousarray(
        W1e.reshape(4, P, ROWG).transpose(1, 0, 2)).astype(NPBF)
    W2b = np.ascontiguousarray(
        W2e.reshape(4, P, ROWG2).transpose(1, 0, 2)).astype(NPBF)

    in_maps = []
    for r in range(R):
        xt = np.zeros((F_IN, NPAD), np.float32)
        xt[:, :PER] = x[r * PER : (r + 1) * PER].T
        in_maps.append({
            "xt0": xt.astype(NPBF),
            "w0e": W0b, "w1e": W1b, "w2e": W2b,
            "b0e": b0e, "b1e": b1e, "b2e": b2e,
            "ilo": ilo_all[r].reshape(ILO_TOT, 16).T.copy(),   # placeholder, fixed below
            "ihi": ihi_all[r].reshape(IHI_TOT, 16).T.copy(),
            "smat": S_all[r], "stmat": St_all[r],
        })
    # proper wrap16 per tile (indices wrapped within each tile's segment)
    for r in range(R):
        lo_w = np.zeros((P, ILO_TOT), np.int16)
        hi_w = np.zeros((P, IHI_TOT), np.int16)
        for t in range(T):
            seg = ilo_all[r, iolo[t] * 16 : iolo[t + 1] * 16]
            lo_w[:, iolo[t] : iolo[t + 1]] = _wrap16(seg)
            seg = ihi_all[r, iohi[t] * 16 : iohi[t + 1] * 16]
            hi_w[:, iohi[t] : iohi[t + 1]] = _wrap16(seg)
        in_maps[r]["ilo"] = lo_w
        in_maps[r]["ihi"] = hi_w

    meta = dict(
        n16_lo=n16_lo.tolist(), n16_hi=n16_hi.tolist(),
        blo=blo.tolist(), bhi=bhi.tolist(), bt=bt.tolist(),
        boff=boff.tolist(), iolo=iolo.tolist(), iohi=iohi.tolist(),
        TOTB=TOTB, ILO_TOT=ILO_TOT, IHI_TOT=IHI_TOT,
        BMAX=int(bt.max()),
    )
    return in_maps, meta


# ---------------------------------------------------------------------------
# Device program
# ---------------------------------------------------------------------------

def build_gat_nc(cfg, meta):
    R, PER, T, NPAD = cfg["R"], cfg["PER"], cfg["T"], cfg["NPAD"]
    F_IN, HID, HEADS, NCLS = cfg["F_IN"], cfg["HID"], cfg["HEADS"], cfg["N_CLASSES"]
    NEG = cfg["NEG"]
    HC = HID * HEADS
    ROWG = 640
    ROWG2 = 128
    SPLIT_T = cfg["SPLIT_T"]
    LO = SPLIT_T * P
    HI = NPAD - LO
    n16_lo, n16_hi = meta["n16_lo"], meta["n16_hi"]
    blo_l, bhi_l, bt_l = meta["blo"], meta["bhi"], meta["bt"]
    boff, iolo, iohi = meta["boff"], meta["iolo"], meta["iohi"]
    TOTB, ILO_TOT, IHI_TOT = meta["TOTB"], meta["ILO_TOT"], meta["IHI_TOT"]
    BMAX = meta["BMAX"]
    ILOMAX = max(n16_lo) // 16
    IHIMAX = max(n16_hi) // 16

    nc = bacc.Bacc("TRN2", target_bir_lowering=False, debug=False,
                   num_devices=R)

    xt0_d = nc.dram_tensor("xt0", [F_IN, NPAD], BF, kind="ExternalInput")
    w0e_d = nc.dram_tensor("w0e", [P, ROWG], BF, kind="ExternalInput")
    w1e_d = nc.dram_tensor("w1e", [P, 4, ROWG], BF, kind="ExternalInput")
    w2e_d = nc.dram_tensor("w2e", [P, 4, ROWG2], BF, kind="ExternalInput")
    b0e_d = nc.dram_tensor("b0e", [P, ROWG], F32, kind="ExternalInput")
    b1e_d = nc.dram_tensor("b1e", [P, ROWG], F32, kind="ExternalInput")
    b2e_d = nc.dram_tensor("b2e", [P, ROWG2], F32, kind="ExternalInput")
    ilo_d = nc.dram_tensor("ilo", [P, ILO_TOT], I16, kind="ExternalInput")
    ihi_d = nc.dram_tensor("ihi", [P, IHI_TOT], I16, kind="ExternalInput")
    smat_d = nc.dram_tensor("smat", [P, TOTB * P], BF, kind="ExternalInput")
    stmat_d = nc.dram_tensor("stmat", [P, TOTB * P], BF, kind="ExternalInput")
    out_d = nc.dram_tensor("out", [PER, NCLS], F32, kind="ExternalOutput")

    rg = [list(range(R))]

    with tile.TileContext(nc) as tc:
        with (
            tc.tile_pool(name="pers", bufs=1) as pers,
            tc.tile_pool(name="sb", bufs=3) as sb,
            tc.tile_pool(name="sbg", bufs=4) as sbg,
            tc.tile_pool(name="psA", bufs=2, space="PSUM") as psA,
            tc.tile_pool(name="psB", bufs=2, space="PSUM") as psB,
            tc.tile_pool(name="psC", bufs=1, space="PSUM") as psC,
            tc.tile_pool(name="psD", bufs=1, space="PSUM") as psD,
            tc.tile_pool(name="dram", bufs=1, space="DRAM") as dram,
        ):
            nc.gpsimd.load_library(library_config.mlp)

            # ---- persistent tiles ----
            ident = pers.tile([P, P], F32)
            make_identity(nc, ident[:])
            xt0 = pers.tile([P, NPAD], BF)
            w0_sb = pers.tile([P, 1, ROWG], BF)
            w1_sb = pers.tile([P, 4, ROWG], BF)
            w2_sb = pers.tile([P, 4, ROWG2], BF)
            b0_sb = pers.tile([P, ROWG], F32)
            b1_sb = pers.tile([P, ROWG], F32)
            b2_sb = pers.tile([P, ROWG2], F32)
            nc.sync.dma_start(xt0[:], xt0_d[:, :])
            nc.sync.dma_start(w0_sb[:, 0, :], w0e_d[:, :])
            nc.sync.dma_start(w1_sb[:], w1e_d[:, :, :])
            nc.sync.dma_start(w2_sb[:], w2e_d[:, :, :])
            nc.sync.dma_start(b0_sb[:], b0e_d[:, :])
            nc.sync.dma_start(b1_sb[:], b1e_d[:, :])
            nc.sync.dma_start(b2_sb[:], b2e_d[:, :])

            # warm the gather ring buffers so trimmed gathers never expose
            # uninitialized SBUF (NaN) to the matmuls
            for _ in range(4):
                gg = sbg.tile([P, BMAX, ROWG], BF, tag="g")
                nc.vector.memset(gg[:].rearrange("p a b -> p (a b)"), 0.0)
                gg2 = sbg.tile([P, BMAX, ROWG2], BF, tag="g2")
                nc.vector.memset(gg2[:].rearrange("p a b -> p (a b)"), 0.0)

            # ---- internal DRAM ----
            hlo = [dram.tile([LO, ROWG], BF, name="hlo0"),
                   dram.tile([LO, ROWG], BF, name="hlo1"),
                   dram.tile([LO, ROWG2], BF, name="hlo2")]
            hhi = [dram.tile([HI, ROWG], BF, name="hhi0"),
                   dram.tile([HI, ROWG], BF, name="hhi1"),
                   dram.tile([HI, ROWG2], BF, name="hhi2")]
            tlo = [dram.tile([R * LO, ROWG], BF, addr_space="Shared", name="tlo0"),
                   dram.tile([R * LO, ROWG], BF, addr_space="Shared", name="tlo1"),
                   dram.tile([R * LO, ROWG2], BF, addr_space="Shared", name="tlo2")]
            thi = [dram.tile([R * HI, ROWG], BF, addr_space="Shared", name="thi0"),
                   dram.tile([R * HI, ROWG], BF, addr_space="Shared", name="thi1"),
                   dram.tile([R * HI, ROWG2], BF, addr_space="Shared", name="thi2")]

            def hext_store(L, t, hsb):
                if t < SPLIT_T:
                    nc.sync.dma_start(hlo[L][t * P : (t + 1) * P, :], hsb[:])
                else:
                    r0 = t * P - LO
                    nc.sync.dma_start(hhi[L][r0 : r0 + P, :], hsb[:])
                if t == SPLIT_T - 1:
                    nc.gpsimd.collective_compute(
                        "AllGather", mybir.AluOpType.bypass,
                        replica_groups=rg, ins=[hlo[L][:, :]],
                        outs=[tlo[L][:, :]])
                if t == T - 1:
                    nc.gpsimd.collective_compute(
                        "AllGather", mybir.AluOpType.bypass,
                        replica_groups=rg, ins=[hhi[L][:, :]],
                        outs=[thi[L][:, :]])

            # ---- layer-0 h_ext prologue: h0 = X @ W0' + b0 ----
            for t in range(T):
                ph = psD.tile([P, ROWG], F32, tag="ph")
                nc.tensor.matmul(ph[:, 0:512],
                                 lhsT=xt0[:, t * P : (t + 1) * P],
                                 rhs=w0_sb[:, 0, 0:512], start=True, stop=True)
                nc.tensor.matmul(ph[:, 512:ROWG],
                                 lhsT=xt0[:, t * P : (t + 1) * P],
                                 rhs=w0_sb[:, 0, 512:ROWG], start=True, stop=True)
                hsb = sb.tile([P, ROWG], BF, tag="hsb640")
                nc.vector.tensor_tensor(hsb[:], ph[:], b0_sb[:],
                                        mybir.AluOpType.add)
                hext_store(0, t, hsb)

            # ---- fused aggregation (+ next-layer h_ext) loops ----
            for L in range(3):
                rowg = ROWG if L < 2 else ROWG2
                nH = HEADS if L < 2 else 1
                ncols = HC if L < 2 else NCLS
                alow = ncols
                adoff = ncols + nH
                gtag = "g" if L < 2 else "g2"
                cls = "" if L < 2 else "2"
                if L == 0:
                    W_nx, b_nx, rowg_nx, KC = w1_sb, b1_sb, ROWG, 4
                elif L == 1:
                    W_nx, b_nx, rowg_nx, KC = w2_sb, b2_sb, ROWG2, 4
                else:
                    W_nx = None

                for t in range(T):
                    nlo, nhi = n16_lo[t], n16_hi[t]
                    blo, bhi = blo_l[t], bhi_l[t]
                    bt = bt_l[t]
                    bo = boff[t]

                    ilo = sb.tile([P, ILOMAX], I16, tag="ilo")
                    ihi = sb.tile([P, IHIMAX], I16, tag="ihi")
                    nc.sync.dma_start(ilo[:, 0 : nlo // 16],
                                      ilo_d[:, iolo[t] : iolo[t + 1]])
                    nc.sync.dma_start(ihi[:, 0 : nhi // 16],
                                      ihi_d[:, iohi[t] : iohi[t + 1]])
                    S_sb = sb.tile([P, BMAX * P], BF, tag="S")
                    St_sb = sb.tile([P, BMAX * P], BF, tag="St")
                    nc.sync.dma_start(S_sb[:, 0 : bt * P],
                                      smat_d[:, bo * P : (bo + bt) * P])
                    nc.sync.dma_start(St_sb[:, 0 : bt * P],
                                      stmat_d[:, bo * P : (bo + bt) * P])
                    loc = sb.tile([P, rowg], BF, tag=f"loc{cls}")
                    if t < SPLIT_T:
                        nc.sync.dma_start(loc[:], hlo[L][t * P : (t + 1) * P, :])
                    else:
                        r0 = t * P - LO
                        nc.sync.dma_start(loc[:], hhi[L][r0 : r0 + P, :])

                    g = sbg.tile([P, BMAX, rowg], BF, tag=gtag)
                    nc.gpsimd.dma_gather(
                        g[:, 0:blo, :], tlo[L][:, :], ilo[:, 0 : nlo // 16],
                        num_idxs=nlo, num_idxs_reg=nlo, elem_size=rowg)
                    nc.gpsimd.dma_gather(
                        g[:, blo:bt, :], thi[L][:, :], ihi[:, 0 : nhi // 16],
                        num_idxs=nhi, num_idxs_reg=nhi, elem_size=rowg)

                    # per-edge al_d via S^T matmuls; pd shares the PSUM tile
                    psmall = psB.tile([P, P], F32, tag="psmall")
                    pad_ps = psmall[:, 0 : bt * nH]
                    pd = psmall[:, BMAX * nH : BMAX * nH + nH]
                    for b in range(bt):
                        nc.tensor.matmul(
                            pad_ps[:, b * nH : (b + 1) * nH],
                            lhsT=St_sb[:, b * P : (b + 1) * P],
                            rhs=loc[:, adoff : adoff + nH],
                            start=True, stop=True)

                    # logits = al_s[src] + al_d[dst] ; clamp ; lrelu ; exp
                    logits = sb.tile([P, BMAX * nH], F32, tag=f"logits{cls}")
                    nc.vector.tensor_tensor(
                        logits[:, 0 : bt * nH].rearrange("p (b h) -> p b h", b=bt),
                        g[:, 0:bt, alow : alow + nH],
                        pad_ps[:].rearrange("p (b h) -> p b h", b=bt),
                        mybir.AluOpType.add)
                    lr = sb.tile([P, BMAX * nH], F32, tag=f"lr{cls}")
                    nc.vector.tensor_scalar_mul(lr[:, 0 : bt * nH],
                                                logits[:, 0 : bt * nH], NEG)
                    nc.vector.tensor_tensor(lr[:, 0 : bt * nH], lr[:, 0 : bt * nH],
                                            logits[:, 0 : bt * nH],
                                            mybir.AluOpType.max)
                    w = sb.tile([P, BMAX * nH], BF, tag=f"w{cls}")
                    nc.scalar.activation(w[:, 0 : bt * nH], lr[:, 0 : bt * nH],
                                         mybir.ActivationFunctionType.Exp)

                    # self-loop: ws = exp(lrelu(al_s_loc + al_d_loc))
                    sl = sb.tile([P, 2 * nH], F32, tag=f"sl{cls}")
                    nc.vector.tensor_tensor(sl[:, 0:nH],
                                            loc[:, alow : alow + nH],
                                            loc[:, adoff : adoff + nH],
                                            mybir.AluOpType.add)
                    nc.vector.tensor_scalar_mul(sl[:, nH : 2 * nH], sl[:, 0:nH], NEG)
                    nc.vector.tensor_tensor(sl[:, nH : 2 * nH], sl[:, nH : 2 * nH],
                                            sl[:, 0:nH], mybir.AluOpType.max)
                    ws = sb.tile([P, nH], F32, tag=f"ws{cls}")
                    nc.scalar.activation(ws[:], sl[:, nH : 2 * nH],
                                         mybir.ActivationFunctionType.Exp)

                    # weight gathered rows in place: g[:,:,0:ncols] *= w
                    nc.vector.tensor_tensor(
                        g[:, 0:bt, 0:ncols].rearrange("p b (h c) -> p b h c", h=nH),
                        g[:, 0:bt, 0:ncols].rearrange("p b (h c) -> p b h c", h=nH),
                        w[:, 0 : bt * nH].rearrange("p (b h) -> p b h", b=bt)
                            .unsqueeze(3)
                            .to_broadcast([P, bt, nH, ncols // nH]),
                        mybir.AluOpType.mult)

                    po = psA.tile([P, HC], F32, tag="po")
                    for b in range(bt):
                        nc.tensor.matmul(po[:, 0:ncols],
                                         lhsT=S_sb[:, b * P : (b + 1) * P],
                                         rhs=g[:, b, 0:ncols],
                                         start=(b == 0), stop=(b == bt - 1))
                        nc.tensor.matmul(pd[:],
                                         lhsT=S_sb[:, b * P : (b + 1) * P],
                                         rhs=w[:, b * nH : (b + 1) * nH],
                                         start=(b == 0), stop=(b == bt - 1))

                    den = sb.tile([P, nH], F32, tag=f"den{cls}")
                    nc.vector.tensor_tensor(den[:], pd[:], ws[:],
                                            mybir.AluOpType.add)
                    rden = sb.tile([P, nH], F32, tag=f"rden{cls}")
                    nc.vector.reciprocal(rden[:], den[:])
                    wr = sb.tile([P, nH], F32, tag=f"wr{cls}")
                    nc.vector.tensor_tensor(wr[:], ws[:], rden[:],
                                            mybir.AluOpType.mult)

                    xn = sb.tile([P, ncols], F32, tag=f"xn{cls}")
                    nc.vector.tensor_tensor(
                        xn[:].rearrange("p (h c) -> p h c", h=nH),
                        po[:, 0:ncols].rearrange("p (h c) -> p h c", h=nH),
                        rden[:].unsqueeze(2).to_broadcast([P, nH, ncols // nH]),
                        mybir.AluOpType.mult)
                    t2 = sb.tile([P, ncols], F32, tag=f"t2{cls}")
                    nc.vector.tensor_tensor(
                        t2[:].rearrange("p (h c) -> p h c", h=nH),
                        loc[:, 0:ncols].rearrange("p (h c) -> p h c", h=nH),
                        wr[:].unsqueeze(2).to_broadcast([P, nH, ncols // nH]),
                        mybir.AluOpType.mult)
                    nc.vector.tensor_tensor(xn[:], xn[:], t2[:],
                                            mybir.AluOpType.add)

                    if L < 2:
                        # ELU+1: xe = relu(x) + exp(-relu(-x)); -1 folded in bias
                        m = sb.tile([P, ncols], F32, tag="elum")
                        nc.scalar.activation(m[:], xn[:],
                                             mybir.ActivationFunctionType.Relu,
                                             scale=-1.0)
                        em = sb.tile([P, ncols], F32, tag="elum")
                        nc.scalar.activation(em[:], m[:],
                                             mybir.ActivationFunctionType.Exp,
                                             scale=-1.0)
                        xe = sb.tile([P, ncols], F32, tag="xe")
                        nc.scalar.activation(xe[:], xn[:],
                                             mybir.ActivationFunctionType.Relu)
                        nc.vector.tensor_tensor(xe[:], xe[:], em[:],
                                                mybir.AluOpType.add)
                        # transpose to feature-major, then next-layer h_ext
                        XtT = sb.tile([P, 4, P], BF, tag="XtT")
                        for c4 in range(4):
                            pt = psC.tile([P, P], F32, tag="pt")
                            nc.tensor.transpose(
                                pt[:], xe[:, c4 * P : (c4 + 1) * P], ident[:])
                            nc.scalar.copy(XtT[:, c4, :], pt[:])
                        ph = psD.tile([P, ROWG], F32, tag="ph")
                        n1 = min(512, rowg_nx)
                        for kc in range(KC):
                            nc.tensor.matmul(ph[:, 0:n1],
                                             lhsT=XtT[:, kc, :],
                                             rhs=W_nx[:, kc, 0:n1],
                                             start=(kc == 0), stop=(kc == KC - 1))
                        if rowg_nx > 512:
                            for kc in range(KC):
                                nc.tensor.matmul(ph[:, 512:rowg_nx],
                                                 lhsT=XtT[:, kc, :],
                                                 rhs=W_nx[:, kc, 512:rowg_nx],
                                                 start=(kc == 0), stop=(kc == KC - 1))
                        hsb = sb.tile([P, rowg_nx], BF, tag=f"hsb{rowg_nx}")
                        nc.vector.tensor_tensor(hsb[:], ph[:, 0:rowg_nx],
                                                b_nx[:], mybir.AluOpType.add)
                        hext_store(L + 1, t, hsb)
                    else:
                        rows = min(P, PER - t * P)
                        nc.sync.dma_start(out_d[t * P : t * P + rows, :],
                                          xn[:rows, 0:NCLS])

    nc.compile()
    nc.m = get_hw_module(nc.m)
    return nc


# ---------------------------------------------------------------------------
# Entry point
# ---------------------------------------------------------------------------

_CACHE = {}


def _get_nc(cfg, meta):
    key = (tuple(sorted(cfg.items())),
           tuple(meta["n16_lo"]), tuple(meta["n16_hi"]))
    if key not in _CACHE:
        _CACHE[key] = build_gat_nc(cfg, meta)
    return _CACHE[key]


def run(inputs, trace=False):
    cfg = real_cfg()
    in_maps, meta = host_prepare(inputs, cfg)
    nc = _get_nc(cfg, meta)
    res = bass_utils.run_bass_kernel_spmd(
        nc, in_maps, core_ids=list(range(cfg["R"])), trace=trace)
    out = np.concatenate([res.results[r]["out"] for r in range(cfg["R"])], axis=0)
    return out[: cfg["N"]], res


def kernel(**inputs) -> np.ndarray:
    out, _ = run(inputs, trace=False)
    return out.astype(np.float32)
